# revision 1
# baseline (speedup 1.0000x reference)
"""Trainium2 Bass kernel for nn_Net_5334349382149.

Key algebraic reduction (exact, holds for every input):
  The network's late MLP consumes x_late = concat([cf, broadcast(pool)], 1)
  and immediately applies InstanceNorm over the CONFIG axis (axis=0):
      h = gelu((x - mean_0(x)) / sqrt(var_0(x) + eps))
  `pool` (the graph read-out) is the SAME vector for every config row, so
  its contribution to x_late @ late_W1 is a per-channel constant across
  configs. A per-channel constant shift is exactly annihilated by the
  axis-0 normalization (the mean shifts by that constant; the variance is
  shift-invariant). The same holds for the config-feature normalization
  offset (also a constant shift across configs). Therefore the reference
  output is mathematically independent of the entire GNN stack; only
      runtime = pred(gelu(norm(gelu(norm(cf' @ W1c)) @ W2)))
  with cf' = cf * (1/(std+1e-4)) column scaling remains. (Verified
  numerically: outputs for random pool vectors agree to ~4e-15.)

Device kernel: the config MLP (1000x24 -> 256 -> 128 -> 1) with the two
axis-0 InstanceNorms, replicated on all 8 cores (no collectives); host
takes core 0's result. bf16 matmuls, fp32 stats.
"""
import os
import sys
import numpy as np

for p in ("/opt/trn_rl_repo", "/opt/pypackages"):
    if p not in sys.path and os.path.isdir(p):
        sys.path.append(p)

import ml_dtypes
import concourse.bass as bass
import concourse.tile as tile
from concourse import bacc, mybir
from concourse.bass_utils import run_bass_kernel_spmd

F32 = mybir.dt.float32
GDT = mybir.dt.bfloat16
AF = mybir.ActivationFunctionType
ALU = mybir.AluOpType
AX = mybir.AxisListType
BF = ml_dtypes.bfloat16

NCORES = 8
HID = 256
CF = 24


def host_prep(d):
    f32 = np.float32
    C_IN = int(np.asarray(d['config_feat']).shape[0])
    CP = ((C_IN + 127) // 128) * 128

    cf_inv = (1.0 / (np.asarray(d['config_feat_std'], f32) + 1e-4)).astype(f32)
    LW1 = np.asarray(d['late_W1'], f32)
    w1c = (LW1[:CF] * cf_inv[:, None]).astype(f32)       # [24, 256]
    cfT = np.zeros((CF, CP), f32)
    cfT[:, :C_IN] = np.asarray(d['config_feat'], f32).T

    m = {
        'cfT': cfT,
        'w1c': w1c,
        'w2la': np.asarray(d['late_W2'], f32)[:128],
        'w2lb': np.asarray(d['late_W2'], f32)[128:],
        'predw': np.asarray(d['pred_W'], f32),
        'predb': np.asarray(d['pred_b'], f32).reshape(1, 1),
    }
    return C_IN, CP, [dict(m) for _ in range(NCORES)]


_prog_cache = {}


def build_program(C, CP):
    nc = bacc.Bacc("TRN2", target_bir_lowering=False, debug=False,
                   num_devices=NCORES)

    def din(name, shape, dt=F32):
        return nc.dram_tensor(name, list(shape), dt, kind="ExternalInput")

    cfT_d = din('cfT', (CF, CP))
    w1c_d = din('w1c', (CF, HID))
    w2l_d = [din('w2la', (128, 128)), din('w2lb', (128, 128))]
    predw_d = din('predw', (128, 1))
    predb_d = din('predb', (1, 1))
    out_d = nc.dram_tensor('out', [1, CP], F32, kind="ExternalOutput")

    cblocks = [(s, min(s + 512, CP)) for s in range(0, CP, 512)]
    NB = len(cblocks)

    with tile.TileContext(nc) as tc, __import__('contextlib').ExitStack() as ctx:
        const = ctx.enter_context(tc.tile_pool(name="const", bufs=1))
        work = ctx.enter_context(tc.tile_pool(name="work", bufs=3))
        col = ctx.enter_context(tc.tile_pool(name="col", bufs=4))
        psum = ctx.enter_context(tc.tile_pool(name="psum", bufs=2, space="PSUM"))

        eps_col = const.tile([128, 1], F32, tag="epsc", name="epsc")
        nc.gpsimd.memset(eps_col[:], 1e-5)
        zero_col = const.tile([128, 1], F32, tag="zeroc", name="zeroc")
        nc.gpsimd.memset(zero_col[:], 0.0)
        nc.const_aps.aps[(F32, 0.0)] = zero_col[:]

        def load_const(dram, tag):
            t = const.tile(list(dram.shape), dram.dtype, tag=tag)
            nc.sync.dma_start(out=t[:], in_=dram[:])
            return t

        cfT = load_const(cfT_d, 'cfT')
        w1c = load_const(w1c_d, 'w1c')
        w2l = [load_const(w2l_d[j], f'w2l{j}') for j in range(2)]
        predw = load_const(predw_d, 'predw')
        predb = load_const(predb_d, 'predb')

        h1 = [const.tile([128, CP], F32, tag=f"h1_{m}", name=f"h1_{m}")
              for m in range(2)]
        h2 = const.tile([128, CP], F32, tag="h2", name="h2")

        def stats_tiles(tagp, n):
            return [work.tile([128, NB], F32, tag=f"{tagp}{m}", name=f"{tagp}{m}")
                    for m in range(n)]

        def norm_factors(st1, st2, nchunk):
            """-> per-chunk (rs, nmr); ACT used only for Sqrt (batched)."""
            sds, mus = [], []
            for m in range(nchunk):
                s1 = col.tile([128, 1], F32, tag="cs1", name="cs1")
                s2c = col.tile([128, 1], F32, tag="cs2", name="cs2")
                nc.vector.tensor_reduce(s1[:], st1[m][:], AX.X, ALU.add)
                nc.vector.tensor_reduce(s2c[:], st2[m][:], AX.X, ALU.add)
                mu = col.tile([128, 1], F32, tag="mu", name="mu")
                nc.vector.tensor_scalar(mu[:], s1[:], 1.0 / C, None, ALU.mult)
                mu2 = col.tile([128, 1], F32, tag="mu2", name="mu2")
                nc.vector.tensor_tensor(mu2[:], mu[:], mu[:], ALU.mult)
                var = col.tile([128, 1], F32, tag="var", name="var")
                nc.vector.scalar_tensor_tensor(var[:], s2c[:], 1.0 / C, mu2[:],
                                               ALU.mult, ALU.subtract)
                mus.append(mu)
                sds.append(var)
            out = []
            for m in range(nchunk):   # batched Sqrt: single ACT table load
                sd = col.tile([128, 1], F32, tag="sd", name="sd")
                nc.scalar.activation(sd[:], sds[m][:], AF.Sqrt, bias=eps_col[:])
                sds[m] = sd
            for m in range(nchunk):
                rs = col.tile([128, 1], F32, tag="rs", name="rs")
                nc.vector.reciprocal(rs[:], sds[m][:])
                nmr = col.tile([128, 1], F32, tag="nmr", name="nmr")
                nc.vector.tensor_scalar(nmr[:], mus[m][:], rs[:], -1.0,
                                        ALU.mult, ALU.mult)
                out.append((rs, nmr))
            return out

        # ---- h1 = gelu(cfgnorm(cf @ w1c)) ----
        st1 = stats_tiles("l1a", 2)
        st2 = stats_tiles("l1b", 2)
        pre1 = [const.tile([128, CP], F32, tag=f"p1_{m}", name=f"p1_{m}")
                for m in range(2)]
        for mc in range(2):
            for bi, (s, e) in enumerate(cblocks):
                w = e - s
                ps = psum.tile([128, 512], F32, tag="mm", name="mm")
                nc.tensor.matmul(ps[:, :w], lhsT=w1c[:, mc * 128:(mc + 1) * 128],
                                 rhs=cfT[:, s:e], start=True, stop=True)

                def cone(a, b, accum, mc=mc, bi=bi, s=s, ps=ps):
                    kw = {'accum_out': st1[mc][:, bi:bi + 1]} if accum else {}
                    nc.vector.tensor_scalar(pre1[mc][:, a:b], ps[:, a - s:b - s],
                                            0.0, 0.0, ALU.add, ALU.add, **kw)
                    if accum:
                        sq = work.tile([128, 512], F32, tag="sqscr", name="sqscr")
                        nc.vector.scalar_tensor_tensor(
                            sq[:, :b - a], pre1[mc][:, a:b], 1.0, pre1[mc][:, a:b],
                            ALU.mult, ALU.mult,
                            accum_out=st2[mc][:, bi:bi + 1])
                if s >= C:
                    cone(s, e, False)
                elif e <= C:
                    cone(s, e, True)
                else:
                    cone(s, C, True)
                    cone(C, e, False)
        f1 = norm_factors(st1, st2, 2)
        for m in range(2):   # batched Gelu: single table load
            nc.scalar.activation(h1[m][:], pre1[m][:], AF.Gelu,
                                 bias=f1[m][1][:], scale=f1[m][0][:])

        # ---- h2 = gelu(cfgnorm(h1 @ w2l)) ----
        st1 = stats_tiles("l2a", 1)
        st2 = stats_tiles("l2b", 1)
        pre2 = const.tile([128, CP], F32, tag="p2", name="p2")
        for bi, (s, e) in enumerate(cblocks):
            w = e - s
            ps = psum.tile([128, 512], F32, tag="mm", name="mm")
            for kc in range(2):
                nc.tensor.matmul(ps[:, :w], lhsT=w2l[kc][:], rhs=h1[kc][:, s:e],
                                 start=(kc == 0), stop=(kc == 1))

            def done(a, b, accum, bi=bi, s=s, ps=ps):
                kw = {'accum_out': st1[0][:, bi:bi + 1]} if accum else {}
                nc.vector.tensor_scalar(pre2[:, a:b], ps[:, a - s:b - s],
                                        0.0, 0.0, ALU.add, ALU.add, **kw)
                if accum:
                    sq = work.tile([128, 512], F32, tag="sqscr", name="sqscr")
                    nc.vector.scalar_tensor_tensor(
                        sq[:, :b - a], pre2[:, a:b], 1.0, pre2[:, a:b],
                        ALU.mult, ALU.mult, accum_out=st2[0][:, bi:bi + 1])
            if s >= C:
                done(s, e, False)
            elif e <= C:
                done(s, e, True)
            else:
                done(s, C, True)
                done(C, e, False)
        f2 = norm_factors(st1, st2, 1)
        nc.scalar.activation(h2[:], pre2[:], AF.Gelu,
                             bias=f2[0][1][:], scale=f2[0][0][:])

        # ---- pred: out = h2^T @ predw + predb ----
        outsb = work.tile([1, CP], F32, tag="outsb", name="outsb")
        for (s, e) in cblocks:
            w = e - s
            ps = psum.tile([1, 512], F32, tag="pred", name="pred")
            nc.tensor.matmul(ps[:, :w], lhsT=predw[:], rhs=h2[:, s:e],
                             start=True, stop=True)
            nc.vector.tensor_scalar(outsb[:, s:e], ps[:, :w], predb[:],
                                    None, ALU.add)
        nc.sync.dma_start(out=out_d[:], in_=outsb[:])

    nc.compile()
    return nc


def kernel(**inputs) -> np.ndarray:
    C, CP, in_maps = host_prep(inputs)
    key = (C, CP)
    if key not in _prog_cache:
        _prog_cache[key] = build_program(C, CP)
    nc = _prog_cache[key]
    res = run_bass_kernel_spmd(nc, in_maps, list(range(NCORES)))
    out = np.asarray(res.results[0]['out']).reshape(-1)[:C]
    return out.astype(np.float32)



# revision 4
# speedup vs baseline: 1.5449x; 1.5449x over previous
"""Trainium2 Bass kernel for nn_Net_5334349382149.

Algebraic reductions (exact for every input):

1. GNN elimination: the late MLP consumes
   x_late = concat([cf_norm, broadcast(pool)], 1) and immediately applies
   InstanceNorm over the config axis (axis=0). `pool` is identical for
   every config row, so its contribution (and the config-normalization's
   constant shift) is a per-channel constant across configs — exactly
   annihilated by the axis-0 normalization. The output therefore depends
   only on config_feat and the late-MLP weights.

2. L1-norm folding: pre1 = cf @ W1c (+ const). Its axis-0 mean/var are
   closed-form in the input covariance:  mu1 = mean_c(cf) @ W1c,
   var1[m] = w_m^T Cov(cf) w_m.  Both are computed exactly on the host
   (~0.3 Mflop numpy) and folded into the weights:
       h1 = gelu(cf @ (W1c * s1) + t1),  s1 = 1/sqrt(var1+eps),
       t1 = -mu1 * s1.
   So the device does no L1 statistics at all.

Device program (replicated on 8 cores, no collectives; host takes core
0's result): bf16 matmuls, fp32 PSUM; a single Gelu ACT-table load
(Copy shares the gelu table, so zero table swaps); L2 InstanceNorm stats
via one-pass bn_stats/bn_aggr on PSUM; 1/sqrt(var+eps) via a quake-seed
+ 2 fused Newton iterations on the vector engine (verified 4.6e-6 rel
err); pred layer written via ACT Copy; pred bias added on host.
"""
import os
import sys
import contextlib
import numpy as np

for p in ("/opt/trn_rl_repo", "/opt/pypackages"):
    if p not in sys.path and os.path.isdir(p):
        sys.path.append(p)

import ml_dtypes
import concourse.bass as bass
import concourse.tile as tile
from concourse import bacc, mybir
from concourse.bass_utils import run_bass_kernel_spmd

F32 = mybir.dt.float32
BF16 = mybir.dt.bfloat16
U32 = mybir.dt.uint32
AF = mybir.ActivationFunctionType
ALU = mybir.AluOpType
BF = ml_dtypes.bfloat16

NCORES = 8
HID = 256
CF = 24
EPS = 1e-5


def host_prep(d):
    f32 = np.float32
    cf = np.asarray(d['config_feat'], f32)          # [C, 24]
    C = cf.shape[0]
    CP = ((C + 127) // 128) * 128

    cf_inv = 1.0 / (np.asarray(d['config_feat_std'], f32) + 1e-4)
    W1c = (np.asarray(d['late_W1'], f32)[:CF] * cf_inv[:, None])  # [24,256]

    # exact L1 InstanceNorm stats from the input covariance (float64)
    cf64 = cf.astype(np.float64)
    W64 = W1c.astype(np.float64)
    mu_cf = cf64.mean(0)                            # [24]
    cc = cf64 - mu_cf
    S = (cc.T @ cc) / C                             # [24,24] biased cov
    mu1 = mu_cf @ W64                               # [256]
    var1 = np.einsum('km,km->m', W64, S @ W64)      # [256]
    s1 = 1.0 / np.sqrt(var1 + EPS)
    t1 = (-mu1 * s1).astype(f32)                    # [256]
    W1f = (W64 * s1[None, :]).astype(f32)           # [24,256]

    A = np.zeros((CF, CP + HID), BF)                # [24, CP+256]
    A[:, :C] = cf.T.astype(BF)
    A[:, CP:] = W1f.astype(BF)

    W2 = np.asarray(d['late_W2'], f32)              # [256,128]
    B = np.zeros((128, 257), BF)
    B[:, 0:128] = W2[:128].astype(BF)
    B[:, 128:256] = W2[128:].astype(BF)
    B[:, 256:257] = np.asarray(d['pred_W'], f32).astype(BF)

    T1 = np.zeros((128, 2), f32)
    T1[:, 0] = t1[:128]
    T1[:, 1] = t1[128:]

    m = {'A': A, 'B': B, 'T1': T1}
    predb = float(np.asarray(d['pred_b'], f32).reshape(-1)[0])
    return C, CP, predb, [dict(m) for _ in range(NCORES)]


_prog_cache = {}


def build_program(C, CP):
    nc = bacc.Bacc("TRN2", target_bir_lowering=False, debug=False,
                   num_devices=NCORES)

    A_d = nc.dram_tensor('A', [CF, CP + HID], BF16, kind="ExternalInput")
    B_d = nc.dram_tensor('B', [128, 257], BF16, kind="ExternalInput")
    T1_d = nc.dram_tensor('T1', [128, 2], F32, kind="ExternalInput")
    out_d = nc.dram_tensor('out', [1, CP], F32, kind="ExternalOutput")

    blocks = [(s, min(s + 512, CP)) for s in range(0, CP, 512)]

    with tile.TileContext(nc) as tc, contextlib.ExitStack() as ctx:
        const = ctx.enter_context(tc.tile_pool(name="const", bufs=1))
        work = ctx.enter_context(tc.tile_pool(name="work", bufs=2))
        col = ctx.enter_context(tc.tile_pool(name="col", bufs=2))
        ps1 = ctx.enter_context(tc.tile_pool(name="ps1", bufs=2, space="PSUM"))
        ps2 = ctx.enter_context(tc.tile_pool(name="ps2", bufs=1, space="PSUM"))
        psp = ctx.enter_context(tc.tile_pool(name="psp", bufs=1, space="PSUM"))

        zero_col = const.tile([128, 1], F32, tag="zeroc")
        nc.gpsimd.memset(zero_col[:], 0.0)
        nc.const_aps.aps[(F32, 0.0)] = zero_col[:]

        # warm the Gelu ACT table immediately (overlaps the input DMAs)
        warmo = const.tile([128, 1], F32, tag="warmo")
        nc.scalar.activation(warmo[:], zero_col[:], AF.Gelu,
                             bias=zero_col[:])

        A = const.tile([CF, CP + HID], BF16, tag="A")
        nc.sync.dma_start(out=A[:], in_=A_d[:])
        B = const.tile([128, 257], BF16, tag="B")
        nc.gpsimd.dma_start(out=B[:], in_=B_d[:])
        T1 = const.tile([128, 2], F32, tag="T1")
        nc.gpsimd.dma_start(out=T1[:], in_=T1_d[:])

        # ---- L1: h1[mc] = gelu(cf @ W1f[:, mc] + t1[mc]) ----
        h1 = [const.tile([128, CP], BF16, tag=f"h1_{m}", name=f"h1_{m}")
              for m in range(2)]
        for mc in range(2):
            ps = ps1.tile([128, CP], F32, tag="mm1")
            for (s, e) in blocks:
                nc.tensor.matmul(ps[:, s:e],
                                 lhsT=A[:, CP + mc * 128:CP + (mc + 1) * 128],
                                 rhs=A[:, s:e], start=True, stop=True)
            nc.scalar.activation(h1[mc][:], ps[:], AF.Gelu,
                                 bias=T1[:, mc:mc + 1])

        # ---- L2: pre2 = h1 @ W2 (psum), bn stats over cols < C ----
        ps_2 = ps2.tile([128, CP], F32, tag="mm2")
        bnbuf = work.tile([128, 12], F32, tag="bn")
        for bi, (s, e) in enumerate(blocks):
            for kc in range(2):
                nc.tensor.matmul(ps_2[:, s:e],
                                 lhsT=B[:, kc * 128:(kc + 1) * 128],
                                 rhs=h1[kc][:, s:e],
                                 start=(kc == 0), stop=(kc == 1))
            ve = min(e, C)
            nc.vector.bn_stats(bnbuf[:, 6 * bi:6 * bi + 6], ps_2[:, s:ve])
        mv = col.tile([128, 2], F32, tag="mv")
        nc.vector.bn_aggr(mv[:], bnbuf[:])

        # 1/sqrt(var+eps): quake seed + 2 fused Newton iterations (DVE)
        vpe = col.tile([128, 1], F32, tag="vpe")
        nc.vector.tensor_scalar(vpe[:], mv[:, 1:2], EPS, 0.5, ALU.add,
                                ALU.mult)                      # 0.5*(v+eps)
        v2 = col.tile([128, 1], F32, tag="v2")
        nc.vector.tensor_scalar(v2[:], mv[:, 1:2], EPS, None, ALU.add)
        sdt = col.tile([128, 1], F32, tag="sdt")
        nc.vector.tensor_scalar(sdt[:].bitcast(U32), v2[:].bitcast(U32),
                                1, None, ALU.logical_shift_right)
        y0 = col.tile([128, 1], F32, tag="y0")
        nc.vector.tensor_scalar(y0[:].bitcast(U32), sdt[:].bitcast(U32),
                                float(0x5f3759df), -1.0, ALU.subtract,
                                ALU.mult)
        t1n = col.tile([128, 1], F32, tag="t1n")
        nc.vector.tensor_scalar(t1n[:], y0[:], y0[:], vpe[:], ALU.mult,
                                ALU.mult)
        m1 = col.tile([128, 1], F32, tag="m1")
        nc.vector.tensor_scalar(m1[:], t1n[:], 1.5, y0[:], ALU.subtract,
                                ALU.mult)
        t2 = col.tile([128, 1], F32, tag="t2")
        nc.vector.tensor_scalar(t2[:], m1[:], m1[:], vpe[:], ALU.mult,
                                ALU.mult)
        sc = col.tile([128, 1], F32, tag="sc")
        nc.vector.tensor_scalar(sc[:], t2[:], 1.5, m1[:], ALU.subtract,
                                ALU.mult)                      # 1/sigma
        t2n = col.tile([128, 1], F32, tag="t2n")
        nc.vector.tensor_scalar(t2n[:], mv[:, 0:1], sc[:], -1.0, ALU.mult,
                                ALU.mult)                      # -mean/sigma

        h2 = const.tile([128, CP], BF16, tag="h2")
        nc.scalar.activation(h2[:], ps_2[:], AF.Gelu,
                             bias=t2n[:], scale=sc[:])

        # ---- pred: out = predw^T @ h2 (+predb on host) ----
        ps_p = psp.tile([1, CP], F32, tag="mmp")
        for (s, e) in blocks:
            nc.tensor.matmul(ps_p[:, s:e], lhsT=B[:, 256:257],
                             rhs=h2[:, s:e], start=True, stop=True)
        outsb = work.tile([1, CP], F32, tag="outsb")
        nc.scalar.activation(outsb[:], ps_p[:], AF.Copy)
        nc.sync.dma_start(out=out_d[:], in_=outsb[:])

    nc.compile()
    return nc


def kernel(**inputs) -> np.ndarray:
    C, CP, predb, in_maps = host_prep(inputs)
    key = (C, CP)
    if key not in _prog_cache:
        _prog_cache[key] = build_program(C, CP)
    nc = _prog_cache[key]
    res = run_bass_kernel_spmd(nc, in_maps, list(range(NCORES)))
    out = np.asarray(res.results[0]['out']).reshape(-1)[:C]
    return (out + predb).astype(np.float32)


# revision 5
# speedup vs baseline: 1.8440x; 1.1936x over previous
"""Trainium2 Bass kernel for nn_Net_5334349382149.

Algebraic reductions (exact for every input):

1. GNN elimination: the late MLP consumes
   x_late = concat([cf_norm, broadcast(pool)], 1) and immediately applies
   InstanceNorm over the config axis (axis=0). `pool` is identical for
   every config row, so its contribution (and the config-normalization's
   constant shift) is a per-channel constant across configs — exactly
   annihilated by the axis-0 normalization. The output therefore depends
   only on config_feat and the late-MLP weights.

2. L1-norm folding: pre1 = cf @ W1c (+ const). Its axis-0 mean/var are
   closed-form in the input covariance:  mu1 = mean_c(cf) @ W1c,
   var1[m] = w_m^T Cov(cf) w_m.  Both are computed exactly on the host
   (~0.3 Mflop numpy) and folded into the weights:
       h1 = gelu(cf @ (W1c * s1) + t1),  s1 = 1/sqrt(var1+eps),
       t1 = -mu1 * s1.
   So the device does no L1 statistics at all.

Device program (replicated on 8 cores, no collectives; host takes core
0's result):
  - bf16 matmuls, fp32 PSUM; dummy warm-up matmuls ramp the PE p-state
    (0.65 -> 2.4 GHz) while the input DMAs are in flight.
  - inputs split across the three DMA-capable queues (sync/gpsimd/act)
    so they land in parallel.
  - single Gelu ACT-table load (Copy/Identity share the gelu table, so
    zero table swaps).
  - L2 InstanceNorm stats via one-pass bn_stats/bn_aggr on PSUM.
  - 1/sqrt(var+eps) in SIX DVE ops: quake bit-trick seed stored negated
    (so one Newton iteration lands positive) fused via tensor_scalar's
    dual-ALU form.
  - pred tail pipelined per 512-block: gelu -> matmul -> copy (ACT and
    DVE in parallel) -> per-block DMA. pred bias added on host.
"""
import os
import sys
import contextlib
import numpy as np

for p in ("/opt/trn_rl_repo", "/opt/pypackages"):
    if p not in sys.path and os.path.isdir(p):
        sys.path.append(p)

import ml_dtypes
import concourse.bass as bass
import concourse.tile as tile
from concourse import bacc, mybir
from concourse.bass_utils import run_bass_kernel_spmd

F32 = mybir.dt.float32
BF16 = mybir.dt.bfloat16
U32 = mybir.dt.uint32
I32 = mybir.dt.int32
AF = mybir.ActivationFunctionType
ALU = mybir.AluOpType
BF = ml_dtypes.bfloat16

NCORES = 8
HID = 256
CF = 24
EPS = 1e-5
# quake rsqrt constants: seed_bits = K - (bits(0.5*x) >> 1) - 2^22,
# stored negated via int32 two's complement (see op comments below)
K_PRIME = 0x5F3759DF - 0x00400000          # seed const for half-input
NEG_OFF = float(K_PRIME - (1 << 31))       # = K' - 2^31 (negative)


def host_prep(d):
    f32 = np.float32
    cf = np.asarray(d['config_feat'], f32)          # [C, 24]
    C = cf.shape[0]
    CP = ((C + 127) // 128) * 128

    cf_inv = 1.0 / (np.asarray(d['config_feat_std'], f32) + 1e-4)
    W1c = (np.asarray(d['late_W1'], f32)[:CF] * cf_inv[:, None])  # [24,256]

    # exact L1 InstanceNorm stats from the input covariance (float64)
    cf64 = cf.astype(np.float64)
    W64 = W1c.astype(np.float64)
    mu_cf = cf64.mean(0)
    cc = cf64 - mu_cf
    S = (cc.T @ cc) / C
    mu1 = mu_cf @ W64
    var1 = np.einsum('km,km->m', W64, S @ W64)
    s1 = 1.0 / np.sqrt(var1 + EPS)
    t1 = (-mu1 * s1).astype(f32)
    W1f = (W64 * s1[None, :]).astype(f32)           # [24,256]

    cfT = np.zeros((CF, CP), BF)
    cfT[:, :C] = cf.T.astype(BF)
    half = CP // 2

    W2 = np.asarray(d['late_W2'], f32)              # [256,128]
    B = np.zeros((128, 257), BF)
    B[:, 0:128] = W2[:128].astype(BF)
    B[:, 128:256] = W2[128:].astype(BF)
    B[:, 256:257] = np.asarray(d['pred_W'], f32).astype(BF)

    T1 = np.zeros((128, 2), f32)
    T1[:, 0] = t1[:128]
    T1[:, 1] = t1[128:]

    m = {
        'cf0': np.ascontiguousarray(cfT[:, :half]),
        'cf1': np.ascontiguousarray(cfT[:, half:]),
        'w1t': W1f.astype(BF),
        'B': B,
        'T1': T1,
    }
    predb = float(np.asarray(d['pred_b'], f32).reshape(-1)[0])
    return C, CP, predb, [dict(m) for _ in range(NCORES)]


_prog_cache = {}


def build_program(C, CP):
    nc = bacc.Bacc("TRN2", target_bir_lowering=False, debug=False,
                   num_devices=NCORES)

    half = CP // 2
    cf0_d = nc.dram_tensor('cf0', [CF, half], BF16, kind="ExternalInput")
    cf1_d = nc.dram_tensor('cf1', [CF, half], BF16, kind="ExternalInput")
    w1t_d = nc.dram_tensor('w1t', [CF, HID], BF16, kind="ExternalInput")
    B_d = nc.dram_tensor('B', [128, 257], BF16, kind="ExternalInput")
    T1_d = nc.dram_tensor('T1', [128, 2], F32, kind="ExternalInput")
    out_d = nc.dram_tensor('out', [1, CP], F32, kind="ExternalOutput")

    with tile.TileContext(nc) as tc, contextlib.ExitStack() as ctx:
        const = ctx.enter_context(tc.tile_pool(name="const", bufs=1))
        work = ctx.enter_context(tc.tile_pool(name="work", bufs=2))
        ps1 = ctx.enter_context(tc.tile_pool(name="ps1", bufs=4, space="PSUM"))
        ps2 = ctx.enter_context(tc.tile_pool(name="ps2", bufs=2, space="PSUM"))
        psp = ctx.enter_context(tc.tile_pool(name="psp", bufs=2, space="PSUM"))

        # --- input DMAs first on each DMA-capable queue (parallel) ---
        cf = [const.tile([CF, half], BF16, tag="cf0", name="cf0"),
              const.tile([CF, half], BF16, tag="cf1", name="cf1")]
        w1t = const.tile([CF, HID], BF16, tag="w1t")
        B = const.tile([128, 257], BF16, tag="B")
        T1 = const.tile([128, 2], F32, tag="T1")
        nc.sync.dma_start(out=cf[0][:], in_=cf0_d[:])
        nc.gpsimd.dma_start(out=cf[1][:], in_=cf1_d[:])
        nc.scalar.dma_start(out=w1t[:], in_=w1t_d[:])
        nc.gpsimd.dma_start(out=T1[:], in_=T1_d[:])
        nc.gpsimd.dma_start(out=B[:], in_=B_d[:])

        # --- constants / warm-up (vector memsets so they start early) ---
        zero_col = const.tile([128, 1], F32, tag="zeroc")
        nc.vector.memset(zero_col[:], 0.0)
        nc.const_aps.aps[(F32, 0.0)] = zero_col[:]
        wtile = const.tile([128, 64], BF16, tag="wtile")
        nc.vector.memset(wtile[:], 0.0)

        # warm the Gelu ACT table (overlaps the input DMAs)
        warmo = const.tile([128, 1], F32, tag="warmo")
        nc.scalar.activation(warmo[:], zero_col[:], AF.Gelu,
                             bias=zero_col[:])

        # dummy matmuls: ramp the PE p-state while DMAs land
        def pe_warm(n):
            for _ in range(n):
                pw = ps1.tile([128, 512], F32, tag="mm1", name="pw")
                nc.tensor.matmul(pw[:64, 0:64], lhsT=wtile[:], rhs=wtile[:],
                                 start=True, stop=True)
        pe_warm(16)

        # ---- L1: h1[mc] = gelu(cf @ W1f[:, mc] + t1[mc]) ----
        h1 = [const.tile([128, CP], BF16, tag=f"h1_{m}", name=f"h1_{m}")
              for m in range(2)]
        l1ps = {}
        for b in range(2):
            for mc in range(2):
                ps = ps1.tile([128, 512], F32, tag="mm1", name="l1ps")
                nc.tensor.matmul(ps[:, :],
                                 lhsT=w1t[:, mc * 128:(mc + 1) * 128],
                                 rhs=cf[b][:, :], start=True, stop=True)
                l1ps[(mc, b)] = ps
        for b in range(2):
            for mc in range(2):
                nc.scalar.activation(h1[mc][:, b * 512:(b + 1) * 512],
                                     l1ps[(mc, b)][:], AF.Gelu,
                                     bias=T1[:, mc:mc + 1])

        # ---- L2: pre2 = h1 @ W2 (psum); bn stats over cols < C ----
        bnbuf = work.tile([128, 12], F32, tag="bn")
        l2ps = []
        for b in range(2):
            ps_2 = ps2.tile([128, 512], F32, tag="mm2", name="ps_2")
            for kc in range(2):
                nc.tensor.matmul(ps_2[:, :],
                                 lhsT=B[:, kc * 128:(kc + 1) * 128],
                                 rhs=h1[kc][:, b * 512:(b + 1) * 512],
                                 start=(kc == 0), stop=(kc == 1))
            w = min(512, C - b * 512)
            nc.vector.bn_stats(bnbuf[:, 6 * b:6 * b + 6], ps_2[:, 0:w])
            l2ps.append(ps_2)
        mv = work.tile([128, 2], F32, tag="mv")
        nc.vector.bn_aggr(mv[:], bnbuf[:])

        # keep the PE hot through the stats/rsqrt gap
        pe_warm(10)

        # ---- 1/sqrt(var+eps): 6-op quake/Newton chain on DVE ----
        vpe = work.tile([128, 1], F32, tag="vpe")
        nc.vector.tensor_scalar(vpe[:], mv[:, 1:2], EPS, 0.5, ALU.add,
                                ALU.mult)                  # 0.5*(v+eps)
        sdt = work.tile([128, 1], F32, tag="sdt")
        nc.vector.tensor_scalar(sdt[:].bitcast(U32), vpe[:].bitcast(U32),
                                1, None, ALU.logical_shift_right)
        y0n = work.tile([128, 1], F32, tag="y0n")   # negated seed (-y0)
        nc.vector.tensor_scalar(y0n[:].bitcast(I32), sdt[:].bitcast(U32),
                                -NEG_OFF, -1.0, ALU.add, ALU.mult)
        t1n = work.tile([128, 1], F32, tag="t1n")
        nc.vector.tensor_scalar(t1n[:], y0n[:], y0n[:], vpe[:], ALU.mult,
                                ALU.mult)                  # v/2 * y0^2
        sc = work.tile([128, 1], F32, tag="sc")
        nc.vector.tensor_scalar(sc[:], t1n[:], 1.5, y0n[:], ALU.subtract,
                                ALU.mult)   # (t-1.5)(-y0) = y0(1.5-t) > 0
        t2n = work.tile([128, 1], F32, tag="t2n")
        nc.vector.tensor_scalar(t2n[:], mv[:, 0:1], sc[:], -1.0, ALU.mult,
                                ALU.mult)                  # -mean/sigma

        # ---- gelu -> pred -> copy -> dma, pipelined per 512-block ----
        h2 = const.tile([128, CP], BF16, tag="h2")
        outsb = work.tile([1, CP], F32, tag="outsb")
        pps = []
        for b in range(2):
            nc.scalar.activation(h2[:, b * 512:(b + 1) * 512], l2ps[b][:],
                                 AF.Gelu, bias=t2n[:], scale=sc[:])
            ps_p = psp.tile([1, 512], F32, tag="mmp", name="ps_p")
            nc.tensor.matmul(ps_p[:, :], lhsT=B[:, 256:257],
                             rhs=h2[:, b * 512:(b + 1) * 512],
                             start=True, stop=True)
            pps.append(ps_p)
        # copies on two engines in parallel, then per-block DMA
        nc.vector.tensor_scalar(outsb[:, 0:512], pps[0][:], 0.0, None,
                                ALU.add)
        nc.sync.dma_start(out=out_d[:, 0:512], in_=outsb[:, 0:512])
        nc.scalar.activation(outsb[:, 512:1024], pps[1][:], AF.Copy)
        nc.gpsimd.dma_start(out=out_d[:, 512:1024], in_=outsb[:, 512:1024])

    nc.compile()
    return nc


def kernel(**inputs) -> np.ndarray:
    C, CP, predb, in_maps = host_prep(inputs)
    key = (C, CP)
    if key not in _prog_cache:
        _prog_cache[key] = build_program(C, CP)
    nc = _prog_cache[key]
    res = run_bass_kernel_spmd(nc, in_maps, list(range(NCORES)))
    out = np.asarray(res.results[0]['out']).reshape(-1)[:C]
    return (out + predb).astype(np.float32)
